# revision 1
# baseline (speedup 1.0000x reference)
"""Trainium2 Bass kernel for causal MultiHeadAttention.

Problem: x[4, 2048, 768], 12 heads x 64 dim, causal, scale = 768**-0.5,
y = softmax(mask(q @ k.T * scale)) @ v  (concat heads) @ Wp + bp.

Sharding: 8 cores = 4 batches x 2 head-groups (6 heads each).  Each core
computes its batch element's attention for its 6 heads plus the partial
output projection (rows g*384..g*384+384 of Wp); the host sums the two
partials per batch and adds the bias.  No device collectives needed.

Per-core dataflow (all matmuls in fp32r; PSUM fp32):
  1. x [T,768] -> PE-transpose -> xT [768,T]
  2. QT/KT = (Wq|Wk).T @ xT  -> 6 tiles [128,T] (head-pair rows)
     V = xT.T @ Wv -> per s-block [128, 6*65] tiles "[V_h | 1]" (ones col
     makes the PV matmul also emit the softmax denominator row).
  3. per head: ST[s,t] = KT.T-slice @ QT (causal-trimmed), P = exp(ST*scale)
     (diag blocks masked by a 0/1 tile), OT[65,T] += [V|1].T @ P.
     Row 64 of OT = denominators; normalize rows 0:64 into OT_all.
  4. y[t,e] = OT_all.T @ Wp_g -> DMA from PSUM to DRAM.
"""

import sys

if "/opt/trn_rl_repo" not in sys.path:
    sys.path.insert(0, "/opt/trn_rl_repo")

import numpy as np

import concourse.bass as bass
import concourse.mybir as mybir
import concourse.tile as tile
from concourse.bass_utils import run_bass_kernel_spmd

# ---------------------------------------------------------------------------
# This walrus build rejects instructions carrying more than one sem wait
# ("Too many sync wait commands" in setupSyncWait).  Post-pass: move excess
# waits onto preceding same-engine NoOps (the engine stalls identically).
_MAXW = 1


def _split_waits(nc):
    for fn in nc.m.functions:
        for bb in fn.blocks:
            out = []
            for inst in bb.instructions:
                si = getattr(inst, "sync_info", None)
                if (
                    si is not None
                    and si.on_wait
                    and len(si.on_wait) > _MAXW
                    and inst.opcode != "EventSemaphore"
                ):
                    waits = list(si.on_wait)
                    for k, i0 in enumerate(range(_MAXW, len(waits), _MAXW)):
                        out.append(mybir.InstNoOp(
                            name=f"{inst.name}_xw{k}",
                            engine=inst.engine,
                            sync_info=mybir.SyncInfo(
                                on_wait=waits[i0 : i0 + _MAXW], on_update=[]
                            ),
                            bass_nofuse=True,
                        ))
                    inst.sync_info = mybir.SyncInfo(
                        on_wait=waits[:_MAXW], on_update=list(si.on_update)
                    )
                out.append(inst)
            bb.instructions = out
# ---------------------------------------------------------------------------

F32 = mybir.dt.float32
F32R = mybir.dt.float32r
EXP = mybir.ActivationFunctionType.Exp

B, T, C = 4, 2048, 768
H, D = 12, 64
HG = 6            # heads per core (head-group)
N_CORES = 8
SCALE = float(C) ** -0.5


def r(ap):
    return ap.bitcast(F32R)


def build_nc(t=T):
    nt = t // 128          # s-blocks
    ncc = C // 128         # c-chunks (6)
    nch = t // 512         # 512-wide t-chunks
    ndb = 6                # QK d-blocks of 128 (3 q head-pairs + 3 k)

    nc = bass.Bass("TRN2", target_bir_lowering=False, debug=False,
                   num_devices=N_CORES)
    x_d = nc.dram_tensor("x", [t, C], F32, kind="ExternalInput")
    wqk_d = nc.dram_tensor("wqk", [C, 768], F32R, kind="ExternalInput")
    wv_d = nc.dram_tensor("wv", [C, 384], F32R, kind="ExternalInput")
    wp_d = nc.dram_tensor("wp", [384, C], F32R, kind="ExternalInput")
    mask_d = nc.dram_tensor("mask01", [128, 128], F32, kind="ExternalInput")
    ident_d = nc.dram_tensor("ident", [128, 128], F32, kind="ExternalInput")
    ones_d = nc.dram_tensor("ones64", [1, 64], F32R, kind="ExternalInput")
    y_d = nc.dram_tensor("y", [t, C], F32, kind="ExternalOutput")

    with tile.TileContext(nc) as tc:
        with tc.tile_pool(name="persist", bufs=1) as pp:
            ident = pp.tile([128, 128], F32, name="ident", tag="ident")
            nc.sync.dma_start(ident[:], ident_d[:])
            mask01 = pp.tile([128, 128], F32, name="mask01", tag="mask01")
            nc.sync.dma_start(mask01[:], mask_d[:])
            ones64 = pp.tile([1, 64], F32R, name="ones64", tag="ones64")
            nc.sync.dma_start(ones64[:], ones_d[:])
            # fp32r constant tiles (walrus rejects memset on fp32r directly)
            scr = pp.tile([128, 384], F32, name="scr", tag="scr")
            czero = pp.tile([128, 384], F32R, name="czero", tag="czero")
            cone = pp.tile([128, 6], F32R, name="cone", tag="cone")
            nc.gpsimd.memset(scr[:], 0.0)
            nc.vector.tensor_copy(czero[:], scr[:])
            nc.gpsimd.memset(scr[:, 0:6], 1.0)
            nc.vector.tensor_copy(cone[:], scr[:, 0:6])

            qkt = [pp.tile([128, t], F32R, name=f"qkt{i}", tag=f"qkt{i}") for i in range(ndb)]
            vaug = [pp.tile([128, HG * 65], F32R, name=f"va{i}", tag=f"va{i}") for i in range(nt)]
            otall = [pp.tile([128, t], F32R, name=f"oa{i}", tag=f"oa{i}") for i in range(3)]

            # ---- phases 1+2: transpose x, project QT/KT/V ----
            with (
                tc.tile_pool(name="ph12", bufs=1) as fp,
                tc.tile_pool(name="xst", bufs=6) as xsp,
                tc.tile_pool(name="tps", bufs=2, space="PSUM") as tpp,
                tc.tile_pool(name="qkps", bufs=2, space="PSUM") as qkp,
                tc.tile_pool(name="vps", bufs=2, space="PSUM") as vpp,
            ):
                xt = [fp.tile([128, t], F32R, name=f"xt{i}", tag=f"xt{i}") for i in range(ncc)]
                wqk_sb = [fp.tile([128, 768], F32R, name=f"wqk{i}", tag=f"wqk{i}")
                          for i in range(ncc)]
                wv_sb = [fp.tile([128, 384], F32R, name=f"wv{i}", tag=f"wv{i}")
                         for i in range(ncc)]
                for i in range(ncc):
                    nc.sync.dma_start(wqk_sb[i][:],
                                      wqk_d[i * 128 : (i + 1) * 128, :])
                    nc.sync.dma_start(wv_sb[i][:],
                                      wv_d[i * 128 : (i + 1) * 128, :])

                # transpose x into xt, 512 columns at a time
                for tcg in range((t + 511) // 512):
                    nb = min(4, nt - tcg * 4)
                    xtiles = []
                    for i in range(nb):
                        tb = tcg * 4 + i
                        xs = xsp.tile([128, C], F32, name="xs", tag="xs")
                        nc.sync.dma_start(xs[:], x_d[tb * 128:(tb + 1) * 128, :])
                        xtiles.append(xs)
                    for cc in range(ncc):
                        tp = tpp.tile([128, 512], F32, name="tp", tag="tp")
                        for i in range(nb):
                            nc.tensor.transpose(
                                tp[:, i * 128 : (i + 1) * 128],
                                xtiles[i][:, cc * 128 : (cc + 1) * 128],
                                ident[:],
                            )
                        nc.vector.tensor_copy(
                            xt[cc][:, tcg * 512 : tcg * 512 + nb * 128],
                            tp[:, : nb * 128],
                        )

                # QT / KT: six [128, t] tiles (3 q head-pairs, 3 k head-pairs)
                for db in range(ndb):
                    for tcg in range(nch):
                        qk = qkp.tile([128, 512], F32, name="qk", tag="qk")
                        for cc in range(ncc):
                            nc.tensor.matmul(
                                qk[:],
                                r(wqk_sb[cc][:, db * 128 : (db + 1) * 128]),
                                r(xt[cc][:, tcg * 512 : (tcg + 1) * 512]),
                                start=(cc == 0), stop=(cc == ncc - 1),
                            )
                        nc.vector.tensor_copy(
                            qkt[db][:, tcg * 512 : (tcg + 1) * 512], qk[:]
                        )

                # V: per s-block [128, 6*65] with a ones column per head
                for sb in range(nt):
                    vp = vpp.tile([128, 384], F32, name="vp", tag="vp")
                    for cc in range(ncc):
                        nc.tensor.matmul(
                            vp[:],
                            r(xt[cc][:, sb * 128 : (sb + 1) * 128]),
                            r(wv_sb[cc][:, :]),
                            start=(cc == 0), stop=(cc == ncc - 1),
                        )
                    va = vaug[sb].rearrange("p (h e) -> p h e", e=65)
                    nc.vector.tensor_copy(va[:, :, 64:65], cone[:].unsqueeze(2))
                    nc.scalar.copy(
                        va[:, :, 0:64], vp.rearrange("p (h e) -> p h e", e=64)
                    )

            # ---- phase 3: attention per head ----
            with (
                tc.tile_pool(name="otps", bufs=1, space="PSUM") as otp,
                tc.tile_pool(name="stps", bufs=3, space="PSUM") as stp,
                tc.tile_pool(name="bcps", bufs=1, space="PSUM") as bcpp,
                tc.tile_pool(name="pts", bufs=3) as ptp,
                tc.tile_pool(name="small", bufs=2) as sp,
            ):
                for h in range(HG):
                    ot = otp.tile([65, t], F32, name="ot", tag="ot")
                    hp, prow = h // 2, (h % 2) * 64
                    qt_t, kt_t = qkt[hp], qkt[3 + hp]
                    for tcg in range(nch):
                        c0 = tcg * 512
                        n_sb = min(nt, 4 * tcg + 4)
                        for sb in range(n_sb):
                            t0 = sb * 128
                            off = max(0, t0 - c0)
                            st = stp.tile([128, 512], F32, name="st", tag="st")
                            nc.tensor.matmul(
                                st[:, off:512],
                                r(kt_t[prow : prow + 64, t0 : t0 + 128]),
                                r(qt_t[prow : prow + 64, c0 + off : c0 + 512]),
                                start=True, stop=True,
                            )
                            pt = ptp.tile([128, 512], F32R, name="pt", tag="pt")
                            if off:
                                nc.vector.tensor_copy(pt[:, 0:off],
                                                      czero[:, 0:off])
                            nc.scalar.activation(
                                pt[:, off:512], st[:, off:512], EXP, scale=SCALE
                            )
                            if t0 >= c0:
                                nc.vector.tensor_mul(
                                    pt[:, off : off + 128],
                                    pt[:, off : off + 128],
                                    mask01[:],
                                )
                            nc.tensor.matmul(
                                ot[:, c0 : c0 + 512],
                                r(vaug[sb][:, h * 65 : h * 65 + 65]),
                                r(pt[:]),
                                start=(sb == 0), stop=(sb == n_sb - 1),
                            )
                    # normalize rows 0:64 by row 64 into otall; the
                    # reciprocal row is broadcast across 64 partitions via a
                    # K=1 PE matmul against a ones column.
                    rt = sp.tile([1, t], F32R, name="rt", tag="rt")
                    with nc.allow_low_precision(reason="f32r is 32-bit"):
                        nc.vector.reciprocal(rt[:], ot[64:65, :])
                    for tcg in range(nch):
                        cs = slice(tcg * 512, (tcg + 1) * 512)
                        bcp = bcpp.tile([64, 512], F32, name="bcp", tag="bcp")
                        nc.tensor.matmul(bcp[:], ones64[:], rt[0:1, cs],
                                         start=True, stop=True)
                        bcs = sp.tile([64, 512], F32, name="bcs", tag="bcs")
                        nc.scalar.copy(bcs[:], bcp[:])
                        nc.vector.tensor_mul(
                            otall[hp][prow : prow + 64, cs], ot[0:64, cs],
                            bcs[:],
                        )

            # ---- phase 4: output projection ----
            with (
                tc.tile_pool(name="yps", bufs=4, space="PSUM") as ypp,
                tc.tile_pool(name="ysb", bufs=4) as ysp,
                tc.tile_pool(name="wpp", bufs=1) as wpl,
            ):
                wp_sb = [wpl.tile([128, C], F32R, name=f"wp{i}", tag=f"wp{i}") for i in range(3)]
                for i in range(3):
                    nc.sync.dma_start(
                        wp_sb[i][:], wp_d[i * 128 : (i + 1) * 128, :]
                    )
                for tb in range(nt):
                    for eh in range(2):
                        yp = ypp.tile([128, 384], F32, name="yp", tag="yp")
                        for kc in range(3):
                            nc.tensor.matmul(
                                yp[:],
                                r(otall[kc][:, tb * 128 : (tb + 1) * 128]),
                                r(wp_sb[kc][:, eh * 384 : (eh + 1) * 384]),
                                start=(kc == 0), stop=(kc == 2),
                            )
                        ys = ysp.tile([128, 384], F32, name="ys", tag="ys")
                        nc.scalar.copy(ys[:], yp[:])
                        nc.sync.dma_start(
                            y_d[tb * 128 : (tb + 1) * 128,
                                eh * 384 : (eh + 1) * 384],
                            ys[:],
                        )
    _split_waits(nc)
    return nc


_NC_CACHE = {}


def _get_nc(t=T):
    if t not in _NC_CACHE:
        _NC_CACHE[t] = build_nc(t)
    return _NC_CACHE[t]


def _shard_inputs(x, Wq, Wk, Wv, Wp):
    mask01 = (np.arange(128)[:, None] <= np.arange(128)[None, :]).astype(
        np.float32
    )
    in_maps = []
    for core in range(N_CORES):
        b, g = core // 2, core % 2
        hs = slice(g * HG, (g + 1) * HG)
        wq = np.transpose(Wq[hs], (1, 0, 2)).reshape(C, HG * D)
        wk = np.transpose(Wk[hs], (1, 0, 2)).reshape(C, HG * D)
        wv = np.transpose(Wv[hs], (1, 0, 2)).reshape(C, HG * D)
        in_maps.append({
            "x": np.ascontiguousarray(x[b], dtype=np.float32),
            "wqk": np.ascontiguousarray(
                np.concatenate([wq, wk], axis=1), dtype=np.float32
            ),
            "wv": np.ascontiguousarray(wv, dtype=np.float32),
            "wp": np.ascontiguousarray(
                Wp[g * HG * D : (g + 1) * HG * D], dtype=np.float32
            ),
            "mask01": mask01,
            "ident": np.eye(128, dtype=np.float32),
            "ones64": np.ones((1, 64), dtype=np.float32),
        })
    return in_maps


def kernel(x, Wq, Wk, Wv, Wp, bp, mask):
    assert mask, "kernel hardcodes causal masking"
    x = np.asarray(x, dtype=np.float32)
    nc = _get_nc(T)
    in_maps = _shard_inputs(
        x, np.asarray(Wq), np.asarray(Wk), np.asarray(Wv), np.asarray(Wp)
    )
    res = run_bass_kernel_spmd(nc, in_maps, list(range(N_CORES)))
    bp = np.asarray(bp, dtype=np.float32)
    out = np.empty((B, T, C), dtype=np.float32)
    for b in range(B):
        out[b] = res.results[2 * b]["y"] + res.results[2 * b + 1]["y"] + bp
    return out



# revision 11
# speedup vs baseline: 6.5216x; 6.5216x over previous
"""Trainium2 Bass kernel for causal MultiHeadAttention.

Problem: x[4, 2048, 768], 12 heads x 64 dim, causal, scale = 768**-0.5,
y = softmax(mask(q @ k.T * scale)) @ v  (concat heads) @ Wp + bp.

Sharding: 8 cores = 4 batches x 2 head-groups (6 heads each); core 2b+g
handles batch b, head-group g.  The call is wire-bound (axon-tunneled
devices, ~30-50 MB/s per direction), so every tensor crosses the wire
exactly once in bf16:
  - x: each core uploads HALF of x[b]; a pair AllGather rebuilds it.
  - weights: each core uploads a QUARTER of its head-group's weights; a
    modular quad AllGather ([[0,2,4,6],[1,3,5,7]]) rebuilds them.
  - y: the two head-group partials are summed on-device with a pair
    ReduceScatter (f32), bias is added on-device, and each core outputs
    only its disjoint half of y[b] in bf16.

Per-core dataflow (matmuls in bf16, PSUM f32):
  1. x [T,768] -> PE-transpose -> xT [768,T]
  2. QT/KT = (Wq|Wk).T @ xT  -> 6 tiles [128,T] (head-pair rows)
     V = xT.T @ Wv -> per s-block [128, 6*65] tiles "[V_h | 1]" (ones col
     makes the PV matmul also emit the softmax denominator row).
  3. per head: ST[s,t] = KT.T-slice @ QT (causal-trimmed), P = exp(ST*scale)
     (diag blocks masked by a 0/1 tile), OT[65,T] += [V|1].T @ P.
     Row 64 of OT = denominators; normalize rows 0:64 into OT_all.
  4. ypart[t,e] = OT_all.T @ Wp_g -> internal DRAM (f32), pair
     ReduceScatter -> y half, + bias -> bf16 ExternalOutput.
"""

import sys

if "/opt/trn_rl_repo" not in sys.path:
    sys.path.insert(0, "/opt/trn_rl_repo")

import numpy as np
import ml_dtypes

import jax

import concourse.bass as bass
import concourse.mybir as mybir
import concourse.tile as tile
from concourse.bass_utils import run_bass_kernel_spmd

# Persistent XLA compilation cache: run_bass_kernel_spmd builds a fresh
# jax.jit per call, so without this every call re-runs walrus verify +
# neuronx-cc wrapping (~0.5s).  Harmless if the backend can't
# deserialize (jax falls back to a normal compile).
jax.config.update("jax_compilation_cache_dir", "/tmp/jax_cc_cache")
jax.config.update("jax_persistent_cache_min_compile_time_secs", 0)
jax.config.update("jax_persistent_cache_min_entry_size_bytes", -1)

# ---------------------------------------------------------------------------
# This walrus build rejects instructions carrying more than one sem wait
# ("Too many sync wait commands" in setupSyncWait).  Post-pass: move excess
# waits onto preceding same-engine NoOps (the engine stalls identically).
_MAXW = 1


def _split_waits(nc):
    for fn in nc.m.functions:
        for bb in fn.blocks:
            out = []
            for inst in bb.instructions:
                si = getattr(inst, "sync_info", None)
                if (
                    si is not None
                    and si.on_wait
                    and len(si.on_wait) > _MAXW
                    and inst.opcode != "EventSemaphore"
                ):
                    waits = list(si.on_wait)
                    for k, i0 in enumerate(range(_MAXW, len(waits), _MAXW)):
                        out.append(mybir.InstNoOp(
                            name=f"{inst.name}_xw{k}",
                            engine=inst.engine,
                            sync_info=mybir.SyncInfo(
                                on_wait=waits[i0 : i0 + _MAXW], on_update=[]
                            ),
                            bass_nofuse=True,
                        ))
                    inst.sync_info = mybir.SyncInfo(
                        on_wait=waits[:_MAXW], on_update=list(si.on_update)
                    )
                out.append(inst)
            bb.instructions = out
# ---------------------------------------------------------------------------

F32 = mybir.dt.float32
F32R = mybir.dt.float32r
BF16 = mybir.dt.bfloat16
EXP = mybir.ActivationFunctionType.Exp
BF = ml_dtypes.bfloat16

B, T, C = 4, 2048, 768
H, D = 12, 64
HG = 6            # heads per core (head-group)
N_CORES = 8
SCALE = float(C) ** -0.5
TH = T // 2       # output rows per core


def r(ap):
    return ap.bitcast(F32R)


def build_nc(t=T):
    nt = t // 128          # s-blocks
    ncc = C // 128         # c-chunks (6)
    nch = t // 512         # 512-wide t-chunks
    ndb = 6                # QK d-blocks of 128 (3 q head-pairs + 3 k)
    th = t // 2

    nc = bass.Bass("TRN2", target_bir_lowering=False, debug=False,
                   num_devices=N_CORES)
    xh_d = nc.dram_tensor("xh", [th, C], BF16, kind="ExternalInput")
    wqkq_d = nc.dram_tensor("wqkq", [C // 4, 768], BF16, kind="ExternalInput")
    wvq_d = nc.dram_tensor("wvq", [C // 4, 384], BF16, kind="ExternalInput")
    wpq_d = nc.dram_tensor("wpq", [96, C], BF16, kind="ExternalInput")
    mask_d = nc.dram_tensor("mask01", [128, 128], BF16, kind="ExternalInput")
    ident_d = nc.dram_tensor("ident", [128, 128], BF16, kind="ExternalInput")
    ones_d = nc.dram_tensor("ones128", [1, 128], BF16, kind="ExternalInput")
    bp_d = nc.dram_tensor("bp", [1, C], BF16, kind="ExternalInput")
    y_d = nc.dram_tensor("y", [th, C], BF16, kind="ExternalOutput")

    PAIRS = [[0, 1], [2, 3], [4, 5], [6, 7]]
    QUADS = [[0, 2, 4, 6], [1, 3, 5, 7]]

    with tile.TileContext(nc) as tc:
        with tc.tile_pool(name="dram", bufs=1, space="DRAM") as dp:
            xin = dp.tile([th, C], BF16, name="xin", tag="xin")
            xfull = dp.tile([t, C], BF16, name="xfull", tag="xfull")
            wqki = dp.tile([C // 4, 768], BF16, name="wqki", tag="wqki")
            wqkf = dp.tile([C, 768], BF16, name="wqkf", tag="wqkf")
            wvi = dp.tile([C // 4, 384], BF16, name="wvi", tag="wvi")
            wvf = dp.tile([C, 384], BF16, name="wvf", tag="wvf")
            wpi = dp.tile([96, C], BF16, name="wpi", tag="wpi")
            wpf = dp.tile([384, C], BF16, name="wpf", tag="wpf")
            ypart = dp.tile([t, C], F32, name="ypart", tag="ypart")
            yhalf = dp.tile([th, C], F32, name="yhalf", tag="yhalf")

            # rebuild x[b] and the head-group weights from per-core slices
            nc.gpsimd.dma_start(xin[:], xh_d[:])
            nc.gpsimd.dma_start(wqki[:], wqkq_d[:])
            nc.gpsimd.dma_start(wvi[:], wvq_d[:])
            nc.gpsimd.dma_start(wpi[:], wpq_d[:])
            nc.gpsimd.collective_compute(
                "AllGather", mybir.AluOpType.bypass, replica_groups=PAIRS,
                ins=[xin.opt()], outs=[xfull.opt()],
            )
            nc.gpsimd.collective_compute(
                "AllGather", mybir.AluOpType.bypass, replica_groups=QUADS,
                ins=[wqki.opt()], outs=[wqkf.opt()],
            )
            nc.gpsimd.collective_compute(
                "AllGather", mybir.AluOpType.bypass, replica_groups=QUADS,
                ins=[wvi.opt()], outs=[wvf.opt()],
            )
            nc.gpsimd.collective_compute(
                "AllGather", mybir.AluOpType.bypass, replica_groups=QUADS,
                ins=[wpi.opt()], outs=[wpf.opt()],
            )

            with tc.tile_pool(name="persist", bufs=1) as pp:
                ident = pp.tile([128, 128], BF16, name="ident", tag="ident")
                nc.sync.dma_start(ident[:], ident_d[:])
                mask01 = pp.tile([128, 128], BF16, name="mask01", tag="mask01")
                nc.sync.dma_start(mask01[:], mask_d[:])
                ones128 = pp.tile([1, 128], BF16, name="ones128", tag="ones128")
                nc.sync.dma_start(ones128[:], ones_d[:])
                scr = pp.tile([128, 384], F32, name="scr", tag="scr")
                czero = pp.tile([128, 384], BF16, name="czero", tag="czero")
                cone = pp.tile([128, 6], BF16, name="cone", tag="cone")
                nc.gpsimd.memset(scr[:], 0.0)
                nc.vector.tensor_copy(czero[:], scr[:])
                nc.gpsimd.memset(scr[:, 0:6], 1.0)
                nc.vector.tensor_copy(cone[:], scr[:, 0:6])

                qkt = [pp.tile([128, t], BF16, name=f"qkt{i}", tag=f"qkt{i}") for i in range(ndb)]
                vaug = [pp.tile([128, HG * 65], BF16, name=f"va{i}", tag=f"va{i}") for i in range(nt)]
                otall = [pp.tile([128, t], BF16, name=f"oa{i}", tag=f"oa{i}") for i in range(3)]

                # ---- phases 1+2: transpose x, project QT/KT/V ----
                with (
                    tc.tile_pool(name="ph12", bufs=1) as fp,
                    tc.tile_pool(name="xst", bufs=6) as xsp,
                    tc.tile_pool(name="tps", bufs=2, space="PSUM") as tpp,
                    tc.tile_pool(name="qkps", bufs=2, space="PSUM") as qkp,
                    tc.tile_pool(name="vps", bufs=2, space="PSUM") as vpp,
                ):
                    xt = [fp.tile([128, t], BF16, name=f"xt{i}", tag=f"xt{i}") for i in range(ncc)]
                    wqk_sb = [fp.tile([128, 768], BF16, name=f"wqk{i}", tag=f"wqk{i}")
                              for i in range(ncc)]
                    wv_sb = [fp.tile([128, 384], BF16, name=f"wv{i}", tag=f"wv{i}")
                             for i in range(ncc)]
                    for i in range(ncc):
                        nc.sync.dma_start(wqk_sb[i][:],
                                          wqkf[i * 128 : (i + 1) * 128, :])
                        nc.sync.dma_start(wv_sb[i][:],
                                          wvf[i * 128 : (i + 1) * 128, :])

                    # transpose x into xt, 512 columns at a time
                    for tcg in range((t + 511) // 512):
                        nb = min(4, nt - tcg * 4)
                        xtiles = []
                        for i in range(nb):
                            tb = tcg * 4 + i
                            xs = xsp.tile([128, C], BF16, name="xs", tag="xs")
                            nc.sync.dma_start(xs[:], xfull[tb * 128:(tb + 1) * 128, :])
                            xtiles.append(xs)
                        for cc in range(ncc):
                            tp = tpp.tile([128, 512], BF16, name="tp", tag="tp")
                            for i in range(nb):
                                nc.tensor.transpose(
                                    tp[:, i * 128 : (i + 1) * 128],
                                    xtiles[i][:, cc * 128 : (cc + 1) * 128],
                                    ident[:],
                                )
                            nc.vector.tensor_copy(
                                xt[cc][:, tcg * 512 : tcg * 512 + nb * 128],
                                tp[:, : nb * 128],
                            )

                    # QT / KT: six [128, t] tiles (3 q head-pairs, 3 k pairs)
                    for db in range(ndb):
                        for tcg in range(nch):
                            qk = qkp.tile([128, 512], F32, name="qk", tag="qk")
                            for cc in range(ncc):
                                nc.tensor.matmul(
                                    qk[:],
                                    wqk_sb[cc][:, db * 128 : (db + 1) * 128],
                                    xt[cc][:, tcg * 512 : (tcg + 1) * 512],
                                    start=(cc == 0), stop=(cc == ncc - 1),
                                )
                            nc.vector.tensor_copy(
                                qkt[db][:, tcg * 512 : (tcg + 1) * 512], qk[:]
                            )

                    # V: per s-block [128, 6*65] with a ones column per head
                    for sb in range(nt):
                        vp = vpp.tile([128, 384], F32, name="vp", tag="vp")
                        for cc in range(ncc):
                            nc.tensor.matmul(
                                vp[:],
                                xt[cc][:, sb * 128 : (sb + 1) * 128],
                                wv_sb[cc][:, :],
                                start=(cc == 0), stop=(cc == ncc - 1),
                            )
                        va = vaug[sb].rearrange("p (h e) -> p h e", e=65)
                        nc.vector.tensor_copy(va[:, :, 64:65], cone[:].unsqueeze(2))
                        nc.scalar.copy(
                            va[:, :, 0:64], vp.rearrange("p (h e) -> p h e", e=64)
                        )

                # ---- phase 3: attention per head ----
                with (
                    tc.tile_pool(name="otps", bufs=1, space="PSUM") as otp,
                    tc.tile_pool(name="stps", bufs=3, space="PSUM") as stp,
                    tc.tile_pool(name="bcps", bufs=1, space="PSUM") as bcpp,
                    tc.tile_pool(name="pts", bufs=3) as ptp,
                    tc.tile_pool(name="small", bufs=2) as sp,
                ):
                    for h in range(HG):
                        ot = otp.tile([65, t], F32, name="ot", tag="ot")
                        hp, prow = h // 2, (h % 2) * 64
                        qt_t, kt_t = qkt[hp], qkt[3 + hp]
                        for tcg in range(nch):
                            c0 = tcg * 512
                            n_sb = min(nt, 4 * tcg + 4)
                            for sb in range(n_sb):
                                t0 = sb * 128
                                off = max(0, t0 - c0)
                                st = stp.tile([128, 512], F32, name="st", tag="st")
                                nc.tensor.matmul(
                                    st[:, off:512],
                                    kt_t[prow : prow + 64, t0 : t0 + 128],
                                    qt_t[prow : prow + 64, c0 + off : c0 + 512],
                                    start=True, stop=True,
                                )
                                pt = ptp.tile([128, 512], BF16, name="pt", tag="pt")
                                if off:
                                    nc.vector.tensor_copy(pt[:, 0:off],
                                                          czero[:, 0:off])
                                nc.scalar.activation(
                                    pt[:, off:512], st[:, off:512], EXP, scale=SCALE
                                )
                                if t0 >= c0:
                                    nc.vector.tensor_mul(
                                        pt[:, off : off + 128],
                                        pt[:, off : off + 128],
                                        mask01[:],
                                    )
                                nc.tensor.matmul(
                                    ot[:, c0 : c0 + 512],
                                    vaug[sb][:, h * 65 : h * 65 + 65],
                                    pt[:],
                                    start=(sb == 0), stop=(sb == n_sb - 1),
                                )
                        # normalize rows 0:64 by row 64 into otall; the
                        # reciprocal row is broadcast across 64 partitions via
                        # a K=1 PE matmul against a ones column.
                        rt = sp.tile([1, t], BF16, name="rt", tag="rt")
                        with nc.allow_low_precision(reason="softmax denom bf16"):
                            nc.vector.reciprocal(rt[:], ot[64:65, :])
                        for tcg in range(nch):
                            cs = slice(tcg * 512, (tcg + 1) * 512)
                            bcp = bcpp.tile([64, 512], F32, name="bcp", tag="bcp")
                            nc.tensor.matmul(bcp[:], ones128[:, 0:64],
                                             rt[0:1, cs],
                                             start=True, stop=True)
                            bcs = sp.tile([64, 512], F32, name="bcs", tag="bcs")
                            nc.scalar.copy(bcs[:], bcp[:])
                            with nc.allow_low_precision(reason="bf16 out"):
                                nc.vector.tensor_mul(
                                    otall[hp][prow : prow + 64, cs],
                                    ot[0:64, cs], bcs[:],
                                )

                # ---- phase 4: output projection -> ypart (f32, DRAM) ----
                with (
                    tc.tile_pool(name="yps", bufs=4, space="PSUM") as ypp,
                    tc.tile_pool(name="ysb", bufs=4) as ysp,
                    tc.tile_pool(name="wpp", bufs=1) as wpl,
                ):
                    wp_sb = [wpl.tile([128, C], BF16, name=f"wp{i}", tag=f"wp{i}") for i in range(3)]
                    for i in range(3):
                        nc.sync.dma_start(
                            wp_sb[i][:], wpf[i * 128 : (i + 1) * 128, :]
                        )
                    for tb in range(nt):
                        for eh in range(2):
                            yp = ypp.tile([128, 384], F32, name="yp", tag="yp")
                            for kc in range(3):
                                nc.tensor.matmul(
                                    yp[:],
                                    otall[kc][:, tb * 128 : (tb + 1) * 128],
                                    wp_sb[kc][:, eh * 384 : (eh + 1) * 384],
                                    start=(kc == 0), stop=(kc == 2),
                                )
                            ys = ysp.tile([128, 384], F32, name="ys", tag="ys")
                            nc.scalar.copy(ys[:], yp[:])
                            nc.sync.dma_start(
                                ypart[tb * 128 : (tb + 1) * 128,
                                      eh * 384 : (eh + 1) * 384],
                                ys[:],
                            )

            # ---- phase 5: pair-sum partials, add bias, emit bf16 half ----
            nc.gpsimd.collective_compute(
                "ReduceScatter", mybir.AluOpType.add, replica_groups=PAIRS,
                ins=[ypart.opt()], outs=[yhalf.opt()],
            )
            with (
                tc.tile_pool(name="bps", bufs=2, space="PSUM") as bpp,
                tc.tile_pool(name="bsb", bufs=1) as bsp,
                tc.tile_pool(name="yos", bufs=4) as yop,
            ):
                onesf = bsp.tile([1, 128], BF16, name="onesf", tag="onesf")
                nc.sync.dma_start(onesf[:], ones_d[:])
                bpt = bsp.tile([1, C], BF16, name="bpt", tag="bpt")
                nc.sync.dma_start(bpt[:], bp_d[:])
                bias = bsp.tile([128, C], F32, name="bias", tag="bias")
                for j in range(2):
                    bc = bpp.tile([128, 384], F32, name="bc", tag="bc")
                    nc.tensor.matmul(bc[:], onesf[:],
                                     bpt[:, j * 384 : (j + 1) * 384],
                                     start=True, stop=True)
                    nc.scalar.copy(bias[:, j * 384 : (j + 1) * 384], bc[:])
                for i in range(th // 128):
                    ya = yop.tile([128, C], F32, name="ya", tag="ya")
                    nc.sync.dma_start(ya[:], yhalf[i * 128 : (i + 1) * 128, :])
                    yo = yop.tile([128, C], BF16, name="yo", tag="yo")
                    with nc.allow_low_precision(reason="bf16 output"):
                        nc.vector.tensor_add(yo[:], ya[:], bias[:])
                    nc.sync.dma_start(y_d[i * 128 : (i + 1) * 128, :], yo[:])
    _split_waits(nc)
    return nc


_NC_CACHE = {}


def _get_nc(t=T):
    if t not in _NC_CACHE:
        _NC_CACHE[t] = build_nc(t)
    return _NC_CACHE[t]


def _shard_inputs(x, Wq, Wk, Wv, Wp, bp):
    mask01 = (np.arange(128)[:, None] <= np.arange(128)[None, :]).astype(BF)
    ident = np.eye(128, dtype=BF)
    ones128 = np.ones((1, 128), dtype=BF)
    bp2 = np.asarray(bp, dtype=np.float32).reshape(1, C).astype(BF)
    # per head-group weight matrices (bf16)
    wqk_g, wv_g, wp_g = [], [], []
    for g in range(2):
        hs = slice(g * HG, (g + 1) * HG)
        wq = np.transpose(Wq[hs], (1, 0, 2)).reshape(C, HG * D)
        wk = np.transpose(Wk[hs], (1, 0, 2)).reshape(C, HG * D)
        wqk_g.append(np.concatenate([wq, wk], axis=1).astype(BF))
        wv_g.append(
            np.transpose(Wv[hs], (1, 0, 2)).reshape(C, HG * D).astype(BF)
        )
        wp_g.append(Wp[g * HG * D : (g + 1) * HG * D].astype(BF))
    xb = [np.ascontiguousarray(x[b], dtype=np.float32).astype(BF)
          for b in range(B)]
    in_maps = []
    for core in range(N_CORES):
        b, g = core // 2, core % 2
        q = b  # quad-member index for the weight AllGather
        in_maps.append({
            "xh": np.ascontiguousarray(xb[b][g * TH : (g + 1) * TH]),
            "wqkq": np.ascontiguousarray(
                wqk_g[g][q * 192 : (q + 1) * 192]
            ),
            "wvq": np.ascontiguousarray(wv_g[g][q * 192 : (q + 1) * 192]),
            "wpq": np.ascontiguousarray(wp_g[g][q * 96 : (q + 1) * 96]),
            "mask01": mask01,
            "ident": ident,
            "ones128": ones128,
            "bp": bp2,
        })
    return in_maps


def kernel(x, Wq, Wk, Wv, Wp, bp, mask):
    assert mask, "kernel hardcodes causal masking"
    x = np.asarray(x, dtype=np.float32)
    nc = _get_nc(T)
    in_maps = _shard_inputs(
        x, np.asarray(Wq), np.asarray(Wk), np.asarray(Wv), np.asarray(Wp),
        np.asarray(bp),
    )
    res = run_bass_kernel_spmd(nc, in_maps, list(range(N_CORES)))
    out = np.empty((B, T, C), dtype=np.float32)
    for b in range(B):
        out[b, :TH] = res.results[2 * b]["y"].astype(np.float32)
        out[b, TH:] = res.results[2 * b + 1]["y"].astype(np.float32)
    return out
